# revision 12
# baseline (speedup 1.0000x reference)
"""Causal self-attention (B=4, S=2048, E=1024, D=128, single head) on 8 TRN2 cores.

Sharding: core c = 2*b + h handles batch b; the two cores of a pair split the
causal key range by k-tile parity (h=0 even 128-row k-tiles, h=1 odd). All 8
cores run the *same* instruction stream (uniform SPMD program); per-core
differences live in DRAM data:
  - xt_kv / xt_oth [1024, 1024] bf16: x[b].T columns gathered by s-tile parity
    (own-parity half feeds K/V projection; Q projection uses both)
  - codes [128, 8] f32: 0/1 multipliers for the oth-parity first-visible
    sub-block (h=1 cores zero it, h=0 keep it) -- parity-dependent host data

Everything on the PE runs in bf16 (f32 PSUM accumulation), which runs at full
PE rate at any free width and halves both x DMA and LDWEIGHTS traffic vs the
f32r baseline. Attention is causally *exact* at 128-column granularity via
suffix matmuls: for q-block m (512 queries = 4 parity-local s-tiles) and local
k-tile i, only the visible q-column suffix [128*max(0, i-4m), 512) is computed
by the scores / PV / softmax-denominator matmuls (visibility i <= 4m+j is
parity-independent by construction). PSUM accumulation over nested suffixes is
legal: the i=0 matmul is full-width with start=True (marks the whole 2KB zero
region), later suffixes accumulate, the last carries stop=True.

Diagonal masking runs post-exp on the idle Pool engine (SBUF ops on p, off the
PE/ACT critical path): own-parity blocks zero the upper triangle of the first
visible 128x128 sub-block with affine_select (parity-independent pattern);
oth-parity blocks multiply that sub-block by a per-core 0/1 code column.
Softmax denominators are ones-vector matmuls over the same suffixes, emitted
at block end so the PE loads the `ones` weights once per block.

Each core emits unnormalized PV partials (pvt [128 d, 2048 q] bf16) and
denominators (sums [1, 2048] f32); the host combines the pair:
  out[b] = ((pv0 + pv1) / (s0 + s1)).T  (+ per-core q-column de-permutation)
"""

import os

os.environ.setdefault("MYCRO_LOCAL_CACHE", "1")

import ml_dtypes
import numpy as np

B, S, E, D = 4, 2048, 1024, 128
P = 128
NT = S // P          # 16 global k-tiles per batch
LT = NT // 2         # 8 local (per-core) k-tiles
NQB = 4              # 512-wide query blocks
QBW = 512
NEB = E // P         # 8 e-tiles
SCALE = 1.0 / float(np.sqrt(D))

TRACE = False        # set by test.py for profiling runs
TRACE_KW = {}

_CACHE = {}


def _build_module(reps=1):
    from contextlib import ExitStack

    import concourse.bacc as bacc
    import concourse.mybir as mybir
    import concourse.tile as tile

    f32 = mybir.dt.float32
    bf16 = mybir.dt.bfloat16

    nc = bacc.Bacc("TRN2", target_bir_lowering=False, debug=False, num_devices=8)

    xt_kv = nc.dram_tensor("xt_kv", [E, S // 2], bf16, kind="ExternalInput").ap()
    xt_oth = nc.dram_tensor("xt_oth", [E, S // 2], bf16, kind="ExternalInput").ap()
    wq_d = nc.dram_tensor("wq", [E, D], bf16, kind="ExternalInput").ap()
    wk_d = nc.dram_tensor("wk", [E, D], bf16, kind="ExternalInput").ap()
    wv_d = nc.dram_tensor("wv", [E, D], bf16, kind="ExternalInput").ap()
    bq_d = nc.dram_tensor("bq", [D], f32, kind="ExternalInput").ap()  # pre-scaled
    bk_d = nc.dram_tensor("bk", [D], f32, kind="ExternalInput").ap()
    bv_d = nc.dram_tensor("bv", [D], f32, kind="ExternalInput").ap()
    codes_d = nc.dram_tensor("codes", [P, 1], f32, kind="ExternalInput").ap()
    identb_d = nc.dram_tensor("identb", [P, P], bf16, kind="ExternalInput").ap()
    onesb_d = nc.dram_tensor("onesb", [P, 1], bf16, kind="ExternalInput").ap()
    pvt_d = nc.dram_tensor("pvt", [D, S], bf16, kind="ExternalOutput").ap()
    sums_d = nc.dram_tensor("sums", [1, S], f32, kind="ExternalOutput").ap()

    with tile.TileContext(nc) as tc, ExitStack() as ctx:
        singles = ctx.enter_context(tc.tile_pool(name="singles", bufs=1))
        xpool = ctx.enter_context(tc.tile_pool(name="xpool", bufs=20))
        ppool = ctx.enter_context(tc.tile_pool(name="ppool", bufs=12))
        proj_ps = ctx.enter_context(tc.tile_pool(name="proj_ps", bufs=2, space="PSUM"))
        sc_ps = ctx.enter_context(tc.tile_pool(name="sc_ps", bufs=3, space="PSUM"))
        pv_ps = ctx.enter_context(tc.tile_pool(name="pv_ps", bufs=2, space="PSUM"))
        sum_ps = ctx.enter_context(tc.tile_pool(name="sum_ps", bufs=1, space="PSUM"))

        # ---- constants (ACT HWDGE ring; xt stream owns the SP ring) ----
        w_sb = {}
        for name, dram in (("wk", wk_d), ("wv", wv_d), ("wq", wq_d)):
            t = singles.tile([P, NEB, D], bf16, tag=f"w_{name}")
            nc.scalar.dma_start(t[:], dram.rearrange("(o p) d -> p o d", p=P))
            w_sb[name] = t
        b_sb = {}
        for name, dram in (("bq", bq_d), ("bk", bk_d), ("bv", bv_d)):
            t = singles.tile([P, 1], f32, tag=f"b_{name}")
            nc.scalar.dma_start(t[:], dram.rearrange("(p one) -> p one", one=1))
            b_sb[name] = t
        codes = singles.tile([P, 1], f32, tag="codes")
        nc.scalar.dma_start(codes[:], codes_d[:])
        identb = singles.tile([P, P], bf16, tag="identb")
        nc.scalar.dma_start(identb[:], identb_d[:])
        onesb = singles.tile([P, 1], bf16, tag="onesb")
        nc.scalar.dma_start(onesb[:], onesb_d[:])

        # ---- persistent activations ----
        kt = singles.tile([P, LT, P], bf16, tag="kt")      # K^T  [d, lt, k]
        vt = singles.tile([P, LT, P], bf16, tag="vt")      # V^T  [d, lt, s]
        vn = singles.tile([P, LT, D], bf16, tag="vn")      # V natural [s, lt, d]
        qt = singles.tile([P, 2, LT, P], bf16, tag="qt")   # Q^T [d, half, lt, q]
        pvt_sb = singles.tile([D, S], bf16, tag="pvt_sb")
        sums_sb = singles.tile([1, S], f32, tag="sums_sb")

        def proj_kv_blk(sb):
            """K/V/Q projections for kv-half s-block sb (512 cols)."""
            xts = []
            for eo in range(NEB):
                xtile = xpool.tile([P, QBW], bf16, tag="xt")
                nc.sync.dma_start(
                    xtile[:], xt_kv[eo * P : (eo + 1) * P, sb * QBW : (sb + 1) * QBW]
                )
                xts.append(xtile)
            for name, dst, bias in (("wk", kt, "bk"), ("wv", vt, "bv")):
                ps = proj_ps.tile([P, QBW], f32, tag="ps")
                for eo in range(NEB):
                    nc.tensor.matmul(
                        ps[:],
                        w_sb[name][:, eo, :],
                        xts[eo][:],
                        start=(eo == 0),
                        stop=(eo == NEB - 1),
                    )
                dstv = dst.rearrange("p lt k -> p (lt k)")
                nc.vector.tensor_scalar_add(
                    dstv[:, sb * QBW : (sb + 1) * QBW], ps[:], b_sb[bias][:]
                )
            ps = proj_ps.tile([P, QBW], f32, tag="ps")
            for eo in range(NEB):
                nc.tensor.matmul(
                    ps[:],
                    w_sb["wq"][:, eo, :],
                    xts[eo][:],
                    start=(eo == 0),
                    stop=(eo == NEB - 1),
                )
            qv = qt.rearrange("p h lt k -> p (h lt k)")
            nc.vector.tensor_scalar(
                qv[:, sb * QBW : (sb + 1) * QBW],
                ps[:],
                SCALE,
                b_sb["bq"][:],
                mybir.AluOpType.mult,
                mybir.AluOpType.add,
            )

        def proj_q_oth(t0, ntiles):
            """Q projection for oth-local s-tiles [t0, t0+ntiles)."""
            w = ntiles * P
            xts = []
            for eo in range(NEB):
                xtile = xpool.tile([P, QBW], bf16, tag="xt")
                nc.sync.dma_start(
                    xtile[:, :w], xt_oth[eo * P : (eo + 1) * P, t0 * P : t0 * P + w]
                )
                xts.append(xtile)
            ps = proj_ps.tile([P, QBW], f32, tag="ps")
            for eo in range(NEB):
                nc.tensor.matmul(
                    ps[:, :w],
                    w_sb["wq"][:, eo, :],
                    xts[eo][:, :w],
                    start=(eo == 0),
                    stop=(eo == NEB - 1),
                )
            qv = qt.rearrange("p h lt k -> p (h lt k)")
            off = (S // 2) + t0 * P
            nc.vector.tensor_scalar(
                qv[:, off : off + w],
                ps[:, :w],
                SCALE,
                b_sb["bq"][:],
                mybir.AluOpType.mult,
                mybir.AluOpType.add,
            )

        def v_transpose(lt):
            ps = sc_ps.tile([P, P], bf16, tag="sc")
            nc.tensor.transpose(ps[:], vt[:, lt, :], identb[:])
            nc.vector.tensor_copy(out=vn[:, lt, :], in_=ps[:, :D])

        def attention_blk(half, m):
            """Attention for q-block = {kv,oth}-local s-tiles [4m, 4m+4).

            Suffix-exact causality: local k-tile i touches q-columns
            [128*max(0, i-4m), 512) only (visibility: i <= 4m+j for local
            q-subtile j, parity-independent). The first visible sub-block is
            masked post-exp on the Pool engine: own-parity (half 0) zeroes
            the r_k > r_q triangle; oth-parity multiplies by a 0/1 per-core
            code (h=1 invisible, h=0 fully visible).
            """
            ext = 4 * m + 4
            qv = qt.rearrange("p h lt k -> p (h lt k)")
            qoff = half * (S // 2) + m * QBW
            col0 = (half * 2 + m) * QBW
            pv = pv_ps.tile([P, QBW], f32, tag="pv")
            ptiles = []

            def emit_scores(i):
                q_lo = P * max(0, i - 4 * m)
                sc = sc_ps.tile([P, QBW], f32, tag="sc")
                nc.tensor.matmul(
                    sc[:, q_lo:],
                    kt[:, i, :],
                    qv[:, qoff + q_lo : qoff + QBW],
                    start=True,
                    stop=True,
                )
                p = ppool.tile([P, QBW], bf16, tag="p")
                if half == 1 and i >= 4 * m:
                    # first visible sub-block is parity-data-masked: exp with
                    # per-core bias (h=1: -1e30 -> 0, h=0: 0 -> unchanged)
                    nc.scalar.activation(
                        p[:, q_lo : q_lo + P],
                        sc[:, q_lo : q_lo + P],
                        mybir.ActivationFunctionType.Exp,
                        bias=codes[:, 0:1],
                    )
                    if q_lo + P < QBW:
                        nc.scalar.activation(
                            p[:, q_lo + P :],
                            sc[:, q_lo + P :],
                            mybir.ActivationFunctionType.Exp,
                        )
                else:
                    nc.scalar.activation(
                        p[:, q_lo:], sc[:, q_lo:], mybir.ActivationFunctionType.Exp
                    )
                if half == 0 and i >= 4 * m:
                    # diagonal: keep p where r_q - r_k >= 0, else 0
                    nc.gpsimd.affine_select(
                        out=p[:, q_lo : q_lo + P],
                        in_=p[:, q_lo : q_lo + P],
                        pattern=[[1, P]],
                        compare_op=mybir.AluOpType.is_ge,
                        fill=0.0,
                        base=0,
                        channel_multiplier=-1,
                    )
                ptiles.append((p, q_lo))

            def emit_pv(i):
                p, q_lo = ptiles[i]
                nc.tensor.matmul(
                    pv[:, q_lo:],
                    vn[:, i, :],
                    p[:, q_lo:],
                    start=(i == 0),
                    stop=(i == ext - 1),
                )

            # software-pipeline the PE queue: keep scores 2 iterations ahead of
            # the exp-dependent PV matmuls so the in-order PE never stalls on ACT
            for i in range(ext):
                emit_scores(i)
                if i >= 2:
                    emit_pv(i - 2)
            emit_pv(ext - 2)
            emit_pv(ext - 1)
            sm = sum_ps.tile([1, QBW], f32, tag="sm")
            for idx, (p, q_lo) in enumerate(ptiles):
                nc.tensor.matmul(
                    sm[:, q_lo:],
                    onesb[:, 0:1],
                    p[:, q_lo:],
                    start=(idx == 0),
                    stop=(idx == ext - 1),
                )
            nc.vector.tensor_copy(out=pvt_sb[:, col0 : col0 + QBW], in_=pv[:])
            nc.vector.tensor_copy(out=sums_sb[:, col0 : col0 + QBW], in_=sm[:])
            out_eng = nc.gpsimd if (half * 2 + m) < 3 else nc.sync
            out_eng.dma_start(
                pvt_d[:, col0 : col0 + QBW], pvt_sb[:, col0 : col0 + QBW]
            )
            out_eng.dma_start(
                sums_d[:, col0 : col0 + QBW], sums_sb[:, col0 : col0 + QBW]
            )

        # ---- emission order (priority hint for the scheduler) ----
        for _rep in range(reps):
            proj_kv_blk(0)
            for lt in range(4):
                v_transpose(lt)
            attention_blk(0, 0)
            proj_kv_blk(1)
            for lt in range(4, LT):
                v_transpose(lt)
            proj_q_oth(0, 4)
            attention_blk(0, 1)
            proj_q_oth(4, 4)
            attention_blk(1, 0)
            attention_blk(1, 1)

    nc.compile()
    return nc


def _get_module(reps=1):
    key = ("nc", reps)
    if key not in _CACHE:
        _CACHE[key] = _build_module(reps)
    return _CACHE[key]


def _host_prep(x, Wq, bq, Wk, bk, Wv, bv):
    """Build the 8 per-core input maps plus per-core q-column permutations."""
    x = np.asarray(x, dtype=np.float32)
    bf = ml_dtypes.bfloat16
    in_maps = []
    perms = []
    for c in range(8):
        b, h = divmod(c, 2)
        xt = np.ascontiguousarray(x[b].T)             # [E, S]
        xt3 = xt.reshape(E, NT, P)
        xt_kv = np.ascontiguousarray(
            xt3[:, h::2, :].reshape(E, S // 2).astype(bf)
        )
        xt_oth = np.ascontiguousarray(
            xt3[:, 1 - h :: 2, :].reshape(E, S // 2).astype(bf)
        )
        # oth-half first-visible sub-block exp bias: k-tile g = 2i+h vs
        # q-tile t = 2(4m+j)+1-h at j = i-4m -- visible iff h == 0
        codes = np.full((P, 1), -1.0e30 if h == 1 else 0.0, dtype=np.float32)
        in_maps.append(
            {
                "xt_kv": xt_kv,
                "xt_oth": xt_oth,
                "wq": np.asarray(Wq, np.float32).astype(bf),
                "wk": np.asarray(Wk, np.float32).astype(bf),
                "wv": np.asarray(Wv, np.float32).astype(bf),
                "bq": np.asarray(bq, np.float32) * np.float32(SCALE),
                "bk": np.asarray(bk, np.float32),
                "bv": np.asarray(bv, np.float32),
                "codes": codes,
                "identb": np.eye(P, dtype=bf),
                "onesb": np.ones((P, 1), dtype=bf),
            }
        )
        # storage col -> global q row: cols [0,1024) = kv-local tiles 0..7
        # (global tile 2j+h), cols [1024,2048) = oth tiles (global 2j+1-h)
        perm = np.empty(S, dtype=np.int64)
        for j in range(LT):
            perm[j * P : (j + 1) * P] = (2 * j + h) * P + np.arange(P)
            perm[(LT + j) * P : (LT + j + 1) * P] = (2 * j + 1 - h) * P + np.arange(P)
        perms.append(perm)
    return in_maps, perms


def kernel(x, Wq, bq, Wk, bk, Wv, bv):
    from concourse.bass_utils import run_bass_kernel_spmd

    nc = _get_module()
    in_maps, perms = _host_prep(x, Wq, bq, Wk, bk, Wv, bv)
    res = run_bass_kernel_spmd(
        nc,
        in_maps,
        core_ids=list(range(8)),
        trace=TRACE,
        **TRACE_KW,
    )
    _CACHE["last_result"] = res

    out = np.empty((B, S, D), dtype=np.float32)
    for b in range(B):
        r0, r1 = res.results[2 * b], res.results[2 * b + 1]
        pv = np.zeros((D, S), dtype=np.float64)
        sm = np.zeros((S,), dtype=np.float64)
        for r, perm in ((r0, perms[2 * b]), (r1, perms[2 * b + 1])):
            pv[:, perm] += r["pvt"].astype(np.float64)
            sm[perm] += r["sums"][0].astype(np.float64)
        out[b] = (pv / sm[None, :]).T.astype(np.float32)
    return out


# revision 15
# speedup vs baseline: 1.5515x; 1.5515x over previous
"""Causal self-attention (B=4, S=2048, E=1024, D=128, single head) on 8 TRN2 cores.

Sharding: core c = 2*b + h handles batch b and the QUERIES of s-tile parity h
(8 of 16 128-row tiles, 1024 queries); keys/values cover the full range. Each
core reads only its own parity half of x (2 MB bf16), projects K/V/Q for it,
and receives the partner parity's K^T / V-natural tiles from its pair core
(c ^ 1) over SBUF-to-SBUF remote DMA; softmax then completes on-device and the
host only de-interleaves rows. All 8 cores run the *same* instruction stream;
per-core differences live in DRAM data (xt, codes, peer routing is relative).

Exchange protocol (per rep): after the own-parity projections and V
transposes, 4 remote_dma_broadcast preps (relative dest (0,1) = pair partner;
4 distinct lane slots, 2 SDMA engines each) push kt_own and vn_own into the
partner's double-buffered kt_oth/vn_oth[(rep-1) % 2], then one
trigger_dma(count=None) fires them. Arrival waits (arr_sem >= 8*rep; each prep
gives the dest 16/8 = 2 increments) cannot be expressed inside the Tile
scheduling sim (it models one core, so cross-core increments never come and
the scheduler reports deadlock) -- they are attached to the oth-phase
scores/PV matmuls AFTER the TileContext exits, before nc.compile(); the
move-waits-to-ldweights pass then places them on the weight loads that
actually read the exchanged tiles. No credit/ack channel is needed: the
in-order PE queue plus the data chain (my rep-r oth reads precede my rep-r+1
projections precede my rep-r+1 send precede the partner's rep-r+1 consumption
precede the partner's rep-r+2 send) proves the double buffer is never
overwritten while read. The NRT PSEUDO_SYNC_BARRIER at kernel entry fences
the semaphore clear against early arrivals.

Everything on the PE runs in bf16 (f32 PSUM accumulation) at full PE rate.
Attention is causally exact at 128-column granularity via suffix matmuls over
512-query blocks (visibility i <= 4m+j for both key parities); the first
visible 128x128 sub-block is masked post-exp: own-parity keys via a Pool
affine_select triangle, partner-parity keys via a per-core exp bias of -1e30
(h=0) or 0 (h=1). Softmax denominators are ones-vector matmuls over the same
suffixes. Output: out[q, :] = (pv / sm).T rows for the core's parity.
"""

import os

os.environ.setdefault("MYCRO_LOCAL_CACHE", "1")

import ml_dtypes
import numpy as np

B, S, E, D = 4, 2048, 1024, 128
P = 128
NT = S // P          # 16 global k-tiles per batch
LT = NT // 2         # 8 local (per-core) tiles per parity
QBW = 512
NEB = E // P         # 8 e-tiles
SH = S // 2          # 1024 own-parity positions per core
SCALE = 1.0 / float(np.sqrt(D))

TRACE = False        # set by test.py for profiling runs
TRACE_KW = {}

_CACHE = {}


def _build_module(reps=1):
    from contextlib import ExitStack

    import concourse.bacc as bacc
    import concourse.mybir as mybir
    import concourse.tile as tile

    f32 = mybir.dt.float32
    bf16 = mybir.dt.bfloat16

    nc = bacc.Bacc("TRN2", target_bir_lowering=False, debug=False, num_devices=8)

    xt_d = nc.dram_tensor("xt", [E, SH], bf16, kind="ExternalInput").ap()
    wq_d = nc.dram_tensor("wq", [E, D], bf16, kind="ExternalInput").ap()
    wk_d = nc.dram_tensor("wk", [E, D], bf16, kind="ExternalInput").ap()
    wv_d = nc.dram_tensor("wv", [E, D], bf16, kind="ExternalInput").ap()
    bq_d = nc.dram_tensor("bq", [D], f32, kind="ExternalInput").ap()  # pre-scaled
    bk_d = nc.dram_tensor("bk", [D], f32, kind="ExternalInput").ap()
    bv_d = nc.dram_tensor("bv", [D], f32, kind="ExternalInput").ap()
    codes_d = nc.dram_tensor("codes", [P, 1], f32, kind="ExternalInput").ap()
    onesb_d = nc.dram_tensor("onesb", [P, 1], bf16, kind="ExternalInput").ap()
    identb_d = nc.dram_tensor("identb", [P, P], bf16, kind="ExternalInput").ap()
    pvt_d = nc.dram_tensor("pvt", [D, SH], bf16, kind="ExternalOutput").ap()
    sums_d = nc.dram_tensor("sums", [1, SH], f32, kind="ExternalOutput").ap()

    arr_sem = nc.alloc_semaphore("arr_sem")
    snd_sem = nc.alloc_semaphore("snd_sem")
    # oth-phase matmuls of rep r must wait arr_sem >= 8r, but the Tile
    # scheduling sim would deadlock on a cross-core sem; collect the
    # instructions here and attach the waits after scheduling.
    deferred_waits = []

    with tile.TileContext(nc) as tc, ExitStack() as ctx:
        singles = ctx.enter_context(tc.tile_pool(name="singles", bufs=1))
        xpool = ctx.enter_context(tc.tile_pool(name="xpool", bufs=20))
        ppool = ctx.enter_context(tc.tile_pool(name="ppool", bufs=12))
        proj_ps = ctx.enter_context(tc.tile_pool(name="proj_ps", bufs=2, space="PSUM"))
        sc_ps = ctx.enter_context(tc.tile_pool(name="sc_ps", bufs=3, space="PSUM"))
        pv_ps = ctx.enter_context(tc.tile_pool(name="pv_ps", bufs=2, space="PSUM"))
        sum_ps = ctx.enter_context(tc.tile_pool(name="sum_ps", bufs=1, space="PSUM"))

        # ---- constants (ACT HWDGE ring; xt stream owns the SP ring) ----
        w_sb = {}
        for name, dram in (("wk", wk_d), ("wv", wv_d), ("wq", wq_d)):
            t = singles.tile([P, NEB, D], bf16, tag=f"w_{name}")
            nc.scalar.dma_start(t[:], dram.rearrange("(o p) d -> p o d", p=P))
            w_sb[name] = t
        b_sb = {}
        for name, dram in (("bq", bq_d), ("bk", bk_d), ("bv", bv_d)):
            t = singles.tile([P, 1], f32, tag=f"b_{name}")
            nc.scalar.dma_start(t[:], dram.rearrange("(p one) -> p one", one=1))
            b_sb[name] = t
        codes = singles.tile([P, 1], f32, tag="codes")
        nc.scalar.dma_start(codes[:], codes_d[:])
        identb = singles.tile([P, P], bf16, tag="identb")
        nc.scalar.dma_start(identb[:], identb_d[:])
        onesb = singles.tile([P, 1], bf16, tag="onesb")
        nc.scalar.dma_start(onesb[:], onesb_d[:])

        # ---- persistent activations ----
        kt = singles.tile([P, LT, P], bf16, tag="kt")        # own K^T [d, lt, k]
        vt = singles.tile([P, LT, P], bf16, tag="vt")        # own V^T [d, lt, s]
        vn = singles.tile([P, LT, D], bf16, tag="vn")        # own V nat [s, lt, d]
        qt = singles.tile([P, LT, P], bf16, tag="qt")        # own Q^T [d, lt, q]
        kto = singles.tile([P, 2, LT, P], bf16, tag="kto")   # partner K^T x2 bufs
        vno = singles.tile([P, 2, LT, D], bf16, tag="vno")   # partner V nat x2
        pvt_sb = singles.tile([D, SH], bf16, tag="pvt_sb")
        sums_sb = singles.tile([1, SH], f32, tag="sums_sb")

        def proj_blk(sb):
            """K/V/Q projections for own-parity s-block sb (512 cols)."""
            xts = []
            for eo in range(NEB):
                xtile = xpool.tile([P, QBW], bf16, tag="xt")
                nc.sync.dma_start(
                    xtile[:], xt_d[eo * P : (eo + 1) * P, sb * QBW : (sb + 1) * QBW]
                )
                xts.append(xtile)
            for name, dst, bias in (("wk", kt, "bk"), ("wv", vt, "bv")):
                ps = proj_ps.tile([P, QBW], f32, tag="ps")
                for eo in range(NEB):
                    nc.tensor.matmul(
                        ps[:],
                        w_sb[name][:, eo, :],
                        xts[eo][:],
                        start=(eo == 0),
                        stop=(eo == NEB - 1),
                    )
                dstv = dst.rearrange("p lt k -> p (lt k)")
                nc.vector.tensor_scalar_add(
                    dstv[:, sb * QBW : (sb + 1) * QBW], ps[:], b_sb[bias][:]
                )
            ps = proj_ps.tile([P, QBW], f32, tag="ps")
            for eo in range(NEB):
                nc.tensor.matmul(
                    ps[:],
                    w_sb["wq"][:, eo, :],
                    xts[eo][:],
                    start=(eo == 0),
                    stop=(eo == NEB - 1),
                )
            qv = qt.rearrange("p lt k -> p (lt k)")
            nc.vector.tensor_scalar(
                qv[:, sb * QBW : (sb + 1) * QBW],
                ps[:],
                SCALE,
                b_sb["bq"][:],
                mybir.AluOpType.mult,
                mybir.AluOpType.add,
            )

        def v_transpose(lt):
            ps = sc_ps.tile([P, P], bf16, tag="sc")
            nc.tensor.transpose(ps[:], vt[:, lt, :], identb[:])
            nc.vector.tensor_copy(out=vn[:, lt, :], in_=ps[:, :D])

        def exchange(rep):
            """Push own kt/vn halves into the partner's buffer (rep-1)%2."""
            b = (rep - 1) % 2
            chunks = [
                (kt[:, 0:4, :], kto[:, b, 0:4, :]),
                (kt[:, 4:8, :], kto[:, b, 4:8, :]),
                (vn[:, 0:4, :], vno[:, b, 0:4, :]),
                (vn[:, 4:8, :], vno[:, b, 4:8, :]),
            ]
            for slot, (src, dst) in enumerate(chunks):
                rd = [None] * 8
                rd[slot] = (0, 1)
                nc.gpsimd.remote_dma_broadcast(
                    out_ap=dst,
                    in_ap=src,
                    remote_sem=arr_sem,
                    local_sem=snd_sem,
                    rdests=rd,
                )
            # advertise the (symmetric) remote writes into OUR oth buffers so
            # Tile orders oth-phase readers after the trigger and rep+2
            # triggers after rep's readers
            nc.gpsimd.trigger_dma(
                count=None,
                signals_writable=[kto[:, b], vno[:, b]],
            )

        def attention_blk(m, rep):
            """Attention for own-parity q-block m (s-tiles [4m, 4m+4)) over
            ALL keys: own-parity phase (kt/vn) then partner-parity phase
            (kto/vno buffer (rep-1)%2, gated on arrival post-scheduling).

            Suffix-exact causality: key tile i of either parity touches
            q-columns [128*max(0, i-4m), 512) (visibility i <= 4m+j). The
            first visible sub-block is masked post-exp: own parity zeroes the
            r_k > r_q triangle on the Pool engine; partner parity applies a
            per-core exp bias (h=0: -1e30 -> 0, h=1: 0).
            """
            ext = 4 * m + 4
            b = (rep - 1) % 2
            qv = qt.rearrange("p lt k -> p (lt k)")
            qoff = m * QBW
            pv = pv_ps.tile([P, QBW], f32, tag="pv")
            sm = sum_ps.tile([1, QBW], f32, tag="sm")
            ptiles = []

            def emit_scores(ph, i):
                q_lo = P * max(0, i - 4 * m)
                ktile = kt[:, i, :] if ph == 0 else kto[:, b, i, :]
                sc = sc_ps.tile([P, QBW], f32, tag="sc")
                mm = nc.tensor.matmul(
                    sc[:, q_lo:],
                    ktile,
                    qv[:, qoff + q_lo : qoff + QBW],
                    start=True,
                    stop=True,
                )
                if ph == 1:
                    deferred_waits.append((rep, mm))
                p = ppool.tile([P, QBW], bf16, tag="p")
                if ph == 1 and i >= 4 * m:
                    nc.scalar.activation(
                        p[:, q_lo : q_lo + P],
                        sc[:, q_lo : q_lo + P],
                        mybir.ActivationFunctionType.Exp,
                        bias=codes[:, 0:1],
                    )
                    if q_lo + P < QBW:
                        nc.scalar.activation(
                            p[:, q_lo + P :],
                            sc[:, q_lo + P :],
                            mybir.ActivationFunctionType.Exp,
                        )
                else:
                    nc.scalar.activation(
                        p[:, q_lo:], sc[:, q_lo:], mybir.ActivationFunctionType.Exp
                    )
                if ph == 0 and i >= 4 * m:
                    nc.gpsimd.affine_select(
                        out=p[:, q_lo : q_lo + P],
                        in_=p[:, q_lo : q_lo + P],
                        pattern=[[1, P]],
                        compare_op=mybir.AluOpType.is_ge,
                        fill=0.0,
                        base=0,
                        channel_multiplier=-1,
                    )
                ptiles.append((ph, i, p, q_lo))

            def emit_pv(idx):
                ph, i, p, q_lo = ptiles[idx]
                vtile = vn[:, i, :] if ph == 0 else vno[:, b, i, :]
                mm = nc.tensor.matmul(
                    pv[:, q_lo:],
                    vtile,
                    p[:, q_lo:],
                    start=(idx == 0),
                    stop=(idx == 2 * ext - 1),
                )
                if ph == 1:
                    deferred_waits.append((rep, mm))

            # software-pipeline: scores stay 2 iterations ahead of PV
            n = 0
            for ph in (0, 1):
                for i in range(ext):
                    emit_scores(ph, i)
                    n += 1
                    if n >= 3:
                        emit_pv(n - 3)
            emit_pv(2 * ext - 2)
            emit_pv(2 * ext - 1)
            for idx, (ph, i, p, q_lo) in enumerate(ptiles):
                nc.tensor.matmul(
                    sm[:, q_lo:],
                    onesb[:, 0:1],
                    p[:, q_lo:],
                    start=(idx == 0),
                    stop=(idx == 2 * ext - 1),
                )
            col0 = m * QBW
            nc.vector.tensor_copy(out=pvt_sb[:, col0 : col0 + QBW], in_=pv[:])
            nc.vector.tensor_copy(out=sums_sb[:, col0 : col0 + QBW], in_=sm[:])
            out_eng = nc.gpsimd if m == 0 else nc.sync
            out_eng.dma_start(
                pvt_d[:, col0 : col0 + QBW], pvt_sb[:, col0 : col0 + QBW]
            )
            out_eng.dma_start(
                sums_d[:, col0 : col0 + QBW], sums_sb[:, col0 : col0 + QBW]
            )

        # ---- emission order (priority hint for the scheduler) ----
        for _rep in range(1, reps + 1):
            proj_blk(0)
            for lt in range(4):
                v_transpose(lt)
            proj_blk(1)
            for lt in range(4, LT):
                v_transpose(lt)
            exchange(_rep)
            attention_blk(0, _rep)
            attention_blk(1, _rep)

    # Attach the cross-core arrival gates now that Tile scheduling is done.
    # check=False: Tile may have filled the wait slot already; the
    # generate_event_semaphores pass splits overfull waits, and
    # move_matmul_waits_to_ldweights relocates them onto the LDWEIGHTS that
    # actually reads the exchanged tiles.
    for rep, mm in deferred_waits:
        mm.wait_op(arr_sem, 8 * rep, "sem-ge", check=False)

    nc.compile()
    return nc


def _get_module(reps=1):
    key = ("nc", reps)
    if key not in _CACHE:
        _CACHE[key] = _build_module(reps)
    return _CACHE[key]


def _host_prep(x, Wq, bq, Wk, bk, Wv, bv):
    """Build the 8 per-core input maps plus per-core q-row permutations."""
    x = np.asarray(x, dtype=np.float32)
    bf = ml_dtypes.bfloat16
    in_maps = []
    perms = []
    for c in range(8):
        b, h = divmod(c, 2)
        xt = np.ascontiguousarray(x[b].T)             # [E, S]
        xt3 = xt.reshape(E, NT, P)
        xt_own = np.ascontiguousarray(
            xt3[:, h::2, :].reshape(E, SH).astype(bf)
        )
        # partner-parity phase first-visible sub-block exp bias: key tile
        # g = 2i+(1-h) vs q-tile t = 2(4m+j)+h at j = i-4m: h=0 -> g = t+1
        # (invisible, -1e30), h=1 -> g = t-1 (fully visible, 0)
        codes = np.full((P, 1), -1.0e30 if h == 0 else 0.0, dtype=np.float32)
        in_maps.append(
            {
                "xt": xt_own,
                "wq": np.asarray(Wq, np.float32).astype(bf),
                "wk": np.asarray(Wk, np.float32).astype(bf),
                "wv": np.asarray(Wv, np.float32).astype(bf),
                "bq": np.asarray(bq, np.float32) * np.float32(SCALE),
                "bk": np.asarray(bk, np.float32),
                "bv": np.asarray(bv, np.float32),
                "codes": codes,
                "identb": np.eye(P, dtype=bf),
                "onesb": np.ones((P, 1), dtype=bf),
            }
        )
        # storage col j*128+r -> global q row (2j+h)*128 + r
        perm = np.empty(SH, dtype=np.int64)
        for j in range(LT):
            perm[j * P : (j + 1) * P] = (2 * j + h) * P + np.arange(P)
        perms.append(perm)
    return in_maps, perms


def kernel(x, Wq, bq, Wk, bk, Wv, bv):
    from concourse.bass_utils import run_bass_kernel_spmd

    nc = _get_module()
    in_maps, perms = _host_prep(x, Wq, bq, Wk, bk, Wv, bv)
    res = run_bass_kernel_spmd(
        nc,
        in_maps,
        core_ids=list(range(8)),
        trace=TRACE,
        **TRACE_KW,
    )
    _CACHE["last_result"] = res

    out = np.empty((B, S, D), dtype=np.float32)
    for c in range(8):
        b = c // 2
        r = res.results[c]
        pv = r["pvt"].astype(np.float64)              # [D, SH]
        sm = r["sums"][0].astype(np.float64)          # [SH]
        out[b, perms[c]] = (pv / sm[None, :]).T.astype(np.float32)
    return out
